# revision 8
# baseline (speedup 1.0000x reference)
"""3x3 median blur (replicate padding) on Trainium2, 8-core data parallel.

Problem: noised_image [32,3,512,512] f32 -> median-blurred; cover_image passthrough.

Strategy:
- Shard batch across 8 NeuronCores: 4 images (12 channel-planes) per core.
- Host-side edge-pad each 512x512 plane to 514x514 so the device kernel needs
  exactly one input DMA per plane (no replicate logic on device).
- Per plane ("strip"): partition p holds padded rows 4p..4p+5 (6 rows x 514 f32
  in the free dim), so every tap of the 3x3 window is a free-dim AP offset.
- Exact median-of-9 via an 18-op min/max network, all on the Vector engine
  (this toolchain's GPSIMD lacks TensorTensor; ACT/PE/DMA-CCE can't do f32
  min/max either): vertical sort3 per column (pairs pmn/pmx, then lo/mid/hi),
  then the horizontal combine med3(max3(lo), med3(mid), min3(hi)) with
  sliding-window reuse. The 8-op combine tail is proven minimal by
  exhaustive lattice search.
- fp16 data everywhere: TensorTensor min/max runs in the DVE 2x_1p perf
  mode (2 elems/cycle/partition) when every operand is 2-byte dtype,
  unit-stride, and 4-BYTE-aligned. Quantization error ~5e-4 rel, far under
  the 2e-2 gate (median of rounded values = rounded median, monotone map).
- The 4 sliding-pair ops (mlo/mhi/qmn/qmx) would need an odd-element
  (2B-aligned) operand, which silently drops them to 1x. Instead the
  otherwise-idle Activation engine (no perf modes -> alignment-indifferent)
  produces 1-element-shifted copies lo3s/hi3s/mid3s, keeping all 18 DVE ops
  4B-aligned at 2x. All W+1-wide tensors are padded to 514-elem rows so
  every row start stays 4B-aligned.
- Raw Bass program (explicit semaphores, standalone wait_ge sequencer
  instructions), double-buffered input/output tiles, DMA on the sync (SP)
  engine overlapping compute.
"""
import sys
sys.path.insert(0, '/opt/trn_rl_repo')
from contextlib import ExitStack
import numpy as np

import concourse.bass as bass
import concourse.mybir as mybir
import bass_rust
from concourse import bass_utils

F32 = mybir.dt.float32
F16 = mybir.dt.float16
MIN = mybir.AluOpType.min
MAX = mybir.AluOpType.max

N_CORES = 8
N_CH = 12          # channel-planes per core (4 images x 3 channels)
H = W = 512
HP = WP = 514      # host-padded plane
R = 4              # output rows per partition


def _mk_ap(base, dims, offset):
    c = base.copy()
    c.ap = bass_rust.VecI64Pair(dims)
    c.offset = offset
    return c


def _build_nc(n_ch=N_CH, reps=1, use_gpsimd=False, dt=F16):
    nc = bass.Bass("TRN2")
    x = nc.dram_tensor("x", [n_ch, HP, WP], dt, kind="ExternalInput")
    y = nc.dram_tensor("y", [n_ch, W, W], dt, kind="ExternalOutput")
    DOPS = 13 if use_gpsimd else 18
    GOPS = 5
    with ExitStack() as ctx:
        xs = [ctx.enter_context(nc.sbuf_tensor(f"xs{i}", [128, 6, WP], dt)) for i in range(2)]
        out = [ctx.enter_context(nc.sbuf_tensor(f"outb{i}", [128, R, W], dt)) for i in range(2)]
        pmn = ctx.enter_context(nc.sbuf_tensor("pmn", [128, R, WP], dt))
        pmx = ctx.enter_context(nc.sbuf_tensor("pmx", [128, R, WP], dt))
        lo3 = ctx.enter_context(nc.sbuf_tensor("lo3", [128, R, WP], dt))
        hi3 = ctx.enter_context(nc.sbuf_tensor("hi3", [128, R, WP], dt))
        mid3 = ctx.enter_context(nc.sbuf_tensor("mid3", [128, R, WP], dt))
        mlo = ctx.enter_context(nc.sbuf_tensor("mlo", [128, R, WP], dt))
        mhi = ctx.enter_context(nc.sbuf_tensor("mhi", [128, R, WP], dt))
        qmn = ctx.enter_context(nc.sbuf_tensor("qmn", [128, R, WP], dt))
        qmx = ctx.enter_context(nc.sbuf_tensor("qmx", [128, R, WP], dt))
        lo3s = ctx.enter_context(nc.sbuf_tensor("lo3s", [128, R, WP], dt))
        hi3s = ctx.enter_context(nc.sbuf_tensor("hi3s", [128, R, WP], dt))
        mid3s = ctx.enter_context(nc.sbuf_tensor("mid3s", [128, R, WP], dt))
        A = ctx.enter_context(nc.sbuf_tensor("A", [128, R, W], dt))
        u = ctx.enter_context(nc.sbuf_tensor("u", [128, R, W], dt))
        B = ctx.enter_context(nc.sbuf_tensor("B", [128, R, W], dt))
        fmn = ctx.enter_context(nc.sbuf_tensor("fmn", [128, R, W], dt))
        fmx = ctx.enter_context(nc.sbuf_tensor("fmx", [128, R, W], dt))
        v = ctx.enter_context(nc.sbuf_tensor("v", [128, R, W], dt))
        if use_gpsimd:
            ttb = [ctx.enter_context(nc.sbuf_tensor(f"ttb{i}", [128, R, WP], dt)) for i in range(2)]
            Cb = [ctx.enter_context(nc.sbuf_tensor(f"Cb{i}", [128, R, W], dt)) for i in range(2)]
        else:
            tt = ctx.enter_context(nc.sbuf_tensor("tt", [128, R, WP], dt))
            C = ctx.enter_context(nc.sbuf_tensor("C", [128, R, W], dt))

        sem_in = ctx.enter_context(nc.semaphore())
        sem_out = ctx.enter_context(nc.semaphore())
        sem_dve = ctx.enter_context(nc.semaphore())
        sem_gp = ctx.enter_context(nc.semaphore())
        sem_act = ctx.enter_context(nc.semaphore())

        block = ctx.enter_context(nc.Block())
        n_strips = n_ch * reps

        @block.sync
        def _(sync):
            for i in range(n_strips):
                ch = i % n_ch
                if i >= 2:
                    sync.wait_ge(sem_dve, DOPS * (i - 2) + 2)
                    if use_gpsimd:
                        sync.wait_ge(sem_gp, GOPS * (i - 2) + 3)
                src = _mk_ap(x[ch], [[R * WP, 128], [WP, 6], [1, WP]], ch * HP * WP)
                sync.dma_start(xs[i % 2][:, :, :], src).then_inc(sem_in, 16)
                if i >= 1:
                    oi = i - 1
                    sync.wait_ge(sem_dve, DOPS * (oi + 1))
                    dst = y[oi % n_ch].rearrange("(p r) w -> p r w", r=R)
                    sync.dma_start(dst, out[oi % 2][:, :, :]).then_inc(sem_out, 16)
            oi = n_strips - 1
            sync.wait_ge(sem_dve, DOPS * (oi + 1))
            dst = y[oi % n_ch].rearrange("(p r) w -> p r w", r=R)
            sync.dma_start(dst, out[oi % 2][:, :, :]).then_inc(sem_out, 16)

        if use_gpsimd:
            @block.gpsimd
            def _(gp):
                for i in range(n_strips):
                    xv = xs[i % 2]
                    tv = ttb[i % 2]
                    Cv = Cb[i % 2]
                    gp.wait_ge(sem_in, 16 * (i + 1))
                    t = gp.tensor_tensor(pmx[:, :, :], xv[:, 0:5, :], xv[:, 1:6, :], MAX); t.then_inc(sem_gp, 1)
                    t = gp.tensor_tensor(hi3[:, :, :], pmx[:, :, :], xv[:, 2:6, :], MAX); t.then_inc(sem_gp, 1)
                    if i >= 2:
                        gp.wait_ge(sem_dve, DOPS * (i - 2) + 3)
                    t = gp.tensor_tensor(tv[:, :, :], pmx[:, :, :], xv[:, 2:6, :], MIN); t.then_inc(sem_gp, 1)
                    t = gp.tensor_tensor(mhi[:, :, :], hi3[:, :, 0:W + 1], hi3[:, :, 1:WP], MIN); t.then_inc(sem_gp, 1)
                    if i >= 2:
                        gp.wait_ge(sem_dve, DOPS * (i - 2) + 12)
                    t = gp.tensor_tensor(Cv[:, :, :], mhi[:, :, 0:W], hi3[:, :, 2:WP], MIN); t.then_inc(sem_gp, 1)

        @block.scalar
        def _(act):
            CP = mybir.ActivationFunctionType.Copy
            for i in range(n_strips):
                act.wait_ge(sem_dve, DOPS * i + 3)
                t = act.activation(lo3s[:, :, 0:W + 1], lo3[:, :, 1:WP], CP); t.then_inc(sem_act, 1)
                act.wait_ge(sem_dve, DOPS * i + 4)
                t = act.activation(hi3s[:, :, 0:W + 1], hi3[:, :, 1:WP], CP); t.then_inc(sem_act, 1)
                act.wait_ge(sem_dve, DOPS * i + 6)
                t = act.activation(mid3s[:, :, 0:W + 1], mid3[:, :, 1:WP], CP); t.then_inc(sem_act, 1)

        @block.vector
        def _(vector):
            for i in range(n_strips):
                xv = xs[i % 2]
                ov = out[i % 2]
                if use_gpsimd:
                    tv = ttb[i % 2]
                    Cv = Cb[i % 2]
                vector.wait_ge(sem_in, 16 * (i + 1))
                t = vector.tensor_tensor(pmn[:, :, :], xv[:, 0:4, :], xv[:, 1:5, :], MIN); t.then_inc(sem_dve, 1)
                if not use_gpsimd:
                    t = vector.tensor_tensor(pmx[:, :, :], xv[:, 0:4, :], xv[:, 1:5, :], MAX); t.then_inc(sem_dve, 1)
                t = vector.tensor_tensor(lo3[:, :, :], pmn[:, :, :], xv[:, 2:6, :], MIN); t.then_inc(sem_dve, 1)
                if not use_gpsimd:
                    t = vector.tensor_tensor(hi3[:, :, :], pmx[:, :, :], xv[:, 2:6, :], MAX); t.then_inc(sem_dve, 1)
                    t = vector.tensor_tensor(tt[:, :, :], pmx[:, :, :], xv[:, 2:6, :], MIN); t.then_inc(sem_dve, 1)
                    tv = tt
                else:
                    vector.wait_ge(sem_gp, GOPS * i + 3)
                t = vector.tensor_tensor(mid3[:, :, :], pmn[:, :, :], tv[:, :, :], MAX); t.then_inc(sem_dve, 1)
                vector.wait_ge(sem_act, 3 * i + 1)
                t = vector.tensor_tensor(mlo[:, :, 0:W + 1], lo3[:, :, 0:W + 1], lo3s[:, :, 0:W + 1], MAX); t.then_inc(sem_dve, 1)
                if not use_gpsimd:
                    vector.wait_ge(sem_act, 3 * i + 2)
                    t = vector.tensor_tensor(mhi[:, :, 0:W + 1], hi3[:, :, 0:W + 1], hi3s[:, :, 0:W + 1], MIN); t.then_inc(sem_dve, 1)
                vector.wait_ge(sem_act, 3 * i + 3)
                t = vector.tensor_tensor(qmn[:, :, 0:W + 1], mid3[:, :, 0:W + 1], mid3s[:, :, 0:W + 1], MIN); t.then_inc(sem_dve, 1)
                t = vector.tensor_tensor(qmx[:, :, 0:W + 1], mid3[:, :, 0:W + 1], mid3s[:, :, 0:W + 1], MAX); t.then_inc(sem_dve, 1)
                t = vector.tensor_tensor(A[:, :, :], mlo[:, :, 0:W], lo3[:, :, 2:WP], MAX); t.then_inc(sem_dve, 1)
                if not use_gpsimd:
                    t = vector.tensor_tensor(C[:, :, :], mhi[:, :, 0:W], hi3[:, :, 2:WP], MIN); t.then_inc(sem_dve, 1)
                    Cv = C
                t = vector.tensor_tensor(u[:, :, :], qmx[:, :, 0:W], mid3[:, :, 2:WP], MIN); t.then_inc(sem_dve, 1)
                t = vector.tensor_tensor(B[:, :, :], qmn[:, :, 0:W], u[:, :, :], MAX); t.then_inc(sem_dve, 1)
                t = vector.tensor_tensor(fmn[:, :, :], A[:, :, :], B[:, :, :], MIN); t.then_inc(sem_dve, 1)
                t = vector.tensor_tensor(fmx[:, :, :], A[:, :, :], B[:, :, :], MAX); t.then_inc(sem_dve, 1)
                if use_gpsimd:
                    vector.wait_ge(sem_gp, GOPS * i + 5)
                t = vector.tensor_tensor(v[:, :, :], fmx[:, :, :], Cv[:, :, :], MIN); t.then_inc(sem_dve, 1)
                if i >= 2:
                    vector.wait_ge(sem_out, 16 * (i - 1))
                t = vector.tensor_tensor(ov[:, :, :], fmn[:, :, :], v[:, :, :], MAX); t.then_inc(sem_dve, 1)
    return nc


_NC_CACHE = {}


def _get_nc(use_gpsimd=False):
    key = use_gpsimd
    if key not in _NC_CACHE:
        _NC_CACHE[key] = _build_nc(use_gpsimd=use_gpsimd)
    return _NC_CACHE[key]


def kernel(noised_image, cover_image):
    noised_image = np.asarray(noised_image)
    x16 = np.ascontiguousarray(noised_image, dtype=np.float16)
    nc = _get_nc(use_gpsimd=False)
    per = noised_image.shape[0] // N_CORES  # 4 images per core
    in_maps = []
    for c in range(N_CORES):
        shard = x16[c * per:(c + 1) * per].reshape(N_CH, H, W)
        padded = np.pad(shard, ((0, 0), (1, 1), (1, 1)), mode='edge')
        in_maps.append({"x": np.ascontiguousarray(padded)})
    res = bass_utils.run_bass_kernel_spmd(nc, in_maps, core_ids=list(range(N_CORES)))
    blurred = np.stack([r["y"].reshape(per, 3, H, W) for r in res.results])
    blurred = blurred.reshape(noised_image.shape).astype(np.float32)
    return (blurred, cover_image)



# revision 12
# speedup vs baseline: 1.0637x; 1.0637x over previous
"""3x3 median blur (replicate padding) on Trainium2, 8-core data parallel.

Problem: noised_image [32,3,512,512] f32 -> median-blurred; cover_image passthrough.

Strategy:
- Shard batch across 8 NeuronCores: 4 images (12 channel-planes) per core.
- Host-side edge-pad each 512x512 plane to 514x514 so the device kernel needs
  no replicate logic.
- fp16 data everywhere: TensorTensor min/max runs in the DVE 2x_1p perf mode
  (2 elems/cycle/partition) for 2-byte unit-stride operands. Quantization
  error ~5e-4 rel, far under the 2e-2 gate (median of rounded inputs =
  rounded median, monotone map). Measured: odd (2B-aligned) element offsets
  do NOT drop 2x_1p on this silicon, so sliding-window taps are plain AP
  offsets.
- Exact median-of-9 via the 18-op min/max network (vertical sort3 per column,
  then horizontal med3(max3(lo), med3(mid), min3(hi)) with sliding-window
  reuse; the 8-op combine tail is minimal). All ops on the Vector engine --
  this toolchain's GPSIMD lacks TensorTensor, ACT/PE/DMA-CCE can't do
  min/max, and the custom-DVE path (InstCustomDveAnt) does not pass this
  walrus codegen.
- P=2 planes are batched per instruction to halve per-instruction overhead
  (~200ns each: seq decode + SBUF access latency). To keep every operand a
  legal 3D access pattern, the input is DMA'd as three row-shifted 4-row
  tensors (xa=rows 4p+0..3, xb=+1, xc=+2 per partition p), making each
  vertical tap a contiguous [128, P*4, 514] view. Intermediates are aliased
  (9 live slots) so everything fits SBUF with double-buffered I/O.
- Raw Bass program: explicit semaphores, DMA on the sync (SP) engine
  overlapping compute, 2-superstrip lookahead.
"""
import sys
sys.path.insert(0, '/opt/trn_rl_repo')
from contextlib import ExitStack
import numpy as np

import concourse.bass as bass
import concourse.mybir as mybir
import bass_rust
from concourse import bass_utils

F32 = mybir.dt.float32
F16 = mybir.dt.float16
MIN = mybir.AluOpType.min
MAX = mybir.AluOpType.max

N_CORES = 8
N_CH = 12          # channel-planes per core (4 images x 3 channels)
H = W = 512
HP = WP = 514      # host-padded plane
R = 4              # output rows per partition
P = 2              # planes batched per instruction


def _mk_ap(base, dims, offset):
    c = base.copy()
    c.ap = bass_rust.VecI64Pair(dims)
    c.offset = offset
    return c


def _build_nc(n_ch=N_CH, reps=1, dt=F16, p=P):
    nc = bass.Bass("TRN2")
    x = nc.dram_tensor("x", [n_ch, HP, WP], dt, kind="ExternalInput")
    y = nc.dram_tensor("y", [n_ch, W, W], dt, kind="ExternalOutput")
    DOPS = 18
    PR = p * R
    n_pair = n_ch // p
    with ExitStack() as ctx:
        xa = [ctx.enter_context(nc.sbuf_tensor(f"xa{i}", [128, PR, WP], dt)) for i in range(2)]
        xb = [ctx.enter_context(nc.sbuf_tensor(f"xb{i}", [128, PR, WP], dt)) for i in range(2)]
        xc = [ctx.enter_context(nc.sbuf_tensor(f"xc{i}", [128, PR, WP], dt)) for i in range(2)]
        out = [ctx.enter_context(nc.sbuf_tensor(f"outb{i}", [128, PR, W], dt)) for i in range(2)]
        # 9 aliased working slots, each [128, PR, WP]
        s = [ctx.enter_context(nc.sbuf_tensor(f"s{i}", [128, PR, WP], dt)) for i in range(9)]
        pmn, pmx, tt, lo3, hi3, mid3, mlo, mhi, A = s
        qmn, qmx, u = pmn, pmx, tt       # reused after the vertical stage
        C, B = mlo, mhi                  # reused after ops 9/10
        fmn, fmx, v = lo3, hi3, mid3     # reused in the final med3

        sem_in = ctx.enter_context(nc.semaphore())
        sem_out = ctx.enter_context(nc.semaphore())
        sem_dve = ctx.enter_context(nc.semaphore())

        block = ctx.enter_context(nc.Block())
        n_strips = n_pair * reps

        @block.sync
        def _(sync):
            for i in range(n_strips):
                pr = i % n_pair
                if i >= 2:
                    sync.wait_ge(sem_dve, DOPS * (i - 2) + 5)
                for t_, roff in ((xa, 0), (xb, 1), (xc, 2)):
                    src = _mk_ap(x[0],
                                 [[R * WP, 128], [HP * WP, p], [WP, R], [1, WP]],
                                 (pr * p) * HP * WP + roff * WP)
                    sync.dma_start(t_[i % 2][:, :, :], src).then_inc(sem_in, 16)
                if i >= 1:
                    oi = i - 1
                    sync.wait_ge(sem_dve, DOPS * (oi + 1))
                    dst = _mk_ap(y[0],
                                 [[R * W, 128], [H * W, p], [W, R], [1, W]],
                                 ((oi % n_pair) * p) * H * W)
                    sync.dma_start(dst, out[oi % 2][:, :, :]).then_inc(sem_out, 16)
            oi = n_strips - 1
            sync.wait_ge(sem_dve, DOPS * (oi + 1))
            dst = _mk_ap(y[0],
                         [[R * W, 128], [H * W, p], [W, R], [1, W]],
                         ((oi % n_pair) * p) * H * W)
            sync.dma_start(dst, out[oi % 2][:, :, :]).then_inc(sem_out, 16)

        @block.vector
        def _(vector):
            for i in range(n_strips):
                a, b, c = xa[i % 2], xb[i % 2], xc[i % 2]
                ov = out[i % 2]
                vector.wait_ge(sem_in, 48 * (i + 1))
                # vertical sort3 per column (pairs, then lo/hi/mid)
                t = vector.tensor_tensor(pmn[:, :, :], a[:, :, :], b[:, :, :], MIN); t.then_inc(sem_dve, 1)
                t = vector.tensor_tensor(pmx[:, :, :], a[:, :, :], b[:, :, :], MAX); t.then_inc(sem_dve, 1)
                t = vector.tensor_tensor(lo3[:, :, :], pmn[:, :, :], c[:, :, :], MIN); t.then_inc(sem_dve, 1)
                t = vector.tensor_tensor(hi3[:, :, :], pmx[:, :, :], c[:, :, :], MAX); t.then_inc(sem_dve, 1)
                t = vector.tensor_tensor(tt[:, :, :], pmx[:, :, :], c[:, :, :], MIN); t.then_inc(sem_dve, 1)
                t = vector.tensor_tensor(mid3[:, :, :], pmn[:, :, :], tt[:, :, :], MAX); t.then_inc(sem_dve, 1)
                # horizontal sliding combine
                t = vector.tensor_tensor(mlo[:, :, 0:W + 1], lo3[:, :, 0:W + 1], lo3[:, :, 1:WP], MAX); t.then_inc(sem_dve, 1)
                t = vector.tensor_tensor(mhi[:, :, 0:W + 1], hi3[:, :, 0:W + 1], hi3[:, :, 1:WP], MIN); t.then_inc(sem_dve, 1)
                t = vector.tensor_tensor(A[:, :, 0:W], mlo[:, :, 0:W], lo3[:, :, 2:WP], MAX); t.then_inc(sem_dve, 1)
                t = vector.tensor_tensor(C[:, :, 0:W], mhi[:, :, 0:W], hi3[:, :, 2:WP], MIN); t.then_inc(sem_dve, 1)
                t = vector.tensor_tensor(qmn[:, :, 0:W + 1], mid3[:, :, 0:W + 1], mid3[:, :, 1:WP], MIN); t.then_inc(sem_dve, 1)
                t = vector.tensor_tensor(qmx[:, :, 0:W + 1], mid3[:, :, 0:W + 1], mid3[:, :, 1:WP], MAX); t.then_inc(sem_dve, 1)
                t = vector.tensor_tensor(u[:, :, 0:W], qmx[:, :, 0:W], mid3[:, :, 2:WP], MIN); t.then_inc(sem_dve, 1)
                t = vector.tensor_tensor(B[:, :, 0:W], qmn[:, :, 0:W], u[:, :, 0:W], MAX); t.then_inc(sem_dve, 1)
                # final med3(A, B, C)
                t = vector.tensor_tensor(fmn[:, :, 0:W], A[:, :, 0:W], B[:, :, 0:W], MIN); t.then_inc(sem_dve, 1)
                t = vector.tensor_tensor(fmx[:, :, 0:W], A[:, :, 0:W], B[:, :, 0:W], MAX); t.then_inc(sem_dve, 1)
                t = vector.tensor_tensor(v[:, :, 0:W], fmx[:, :, 0:W], C[:, :, 0:W], MIN); t.then_inc(sem_dve, 1)
                if i >= 2:
                    vector.wait_ge(sem_out, 16 * (i - 1))
                t = vector.tensor_tensor(ov[:, :, :], fmn[:, :, 0:W], v[:, :, 0:W], MAX); t.then_inc(sem_dve, 1)
    return nc


_NC_CACHE = {}


def _get_nc():
    if 'nc' not in _NC_CACHE:
        _NC_CACHE['nc'] = _build_nc()
    return _NC_CACHE['nc']


def kernel(noised_image, cover_image):
    noised_image = np.asarray(noised_image)
    x16 = np.ascontiguousarray(noised_image, dtype=np.float16)
    nc = _get_nc()
    per = noised_image.shape[0] // N_CORES  # 4 images per core
    in_maps = []
    for c in range(N_CORES):
        shard = x16[c * per:(c + 1) * per].reshape(N_CH, H, W)
        padded = np.pad(shard, ((0, 0), (1, 1), (1, 1)), mode='edge')
        in_maps.append({"x": np.ascontiguousarray(padded)})
    res = bass_utils.run_bass_kernel_spmd(nc, in_maps, core_ids=list(range(N_CORES)))
    blurred = np.stack([r["y"].reshape(per, 3, H, W) for r in res.results])
    blurred = blurred.reshape(noised_image.shape).astype(np.float32)
    return (blurred, cover_image)
